# revision 12
# baseline (speedup 1.0000x reference)
"""Trainium2 Bass kernel for nn_AutoReconstruction.

Computes out[b, m] = dot(inputs[b, m, :], W[m, :]) + bias[m]
  inputs: [1024, 2048, 128] f32, W: [2048, 128] f32, bias: [2048] f32
  out:    [1024, 2048] f32

Sharding: batch dim B=1024 split across 8 NeuronCores (128 each); W/bias
replicated. Per-core traffic ~130 MiB -> memory-bound (~360 GB/s/core).

Per-core algorithm (M chunked p-major: m = p*16 + c, so each partition
reads contiguous 8 KB/batch and DMA descriptors stay large):
  - X loaded 4 batches per DMA as SBUF tiles [p=128, nb, c=16, i=128]
  - per batch, DVE computes prod = X*W in one [128, 2048] tensor_mul;
    the i-reduction is split to keep both engines at ~440 us:
      5/9 of batches: DVE tensor_reduce (axis=X) -> acc[:, b, :]
      4/9 of batches: 16x ScalarE activation(accum_out) chunks
  - one broadcast tensor_add applies bias to acc[m_p, b, c] for all b
  - 16 PE transposes flip acc to [b_p, m]; one contiguous 1 MiB out DMA
Measured: ~483 us/core (DVE ~440, ACT ~440, DMA ~365 active @375 GB/s).
fp32 throughout (rel err ~2e-7). Notes: tensor_tensor_reduce faults on
this runtime; PE fp32 matmul/transpose is ~2.5-5 cyc/col, too slow to
host the reduction.
"""

import numpy as np

B, M, I = 1024, 2048, 128
NCORES = 8
BLOC = B // NCORES  # 128 batches per core
C = M // 128        # 16 m-chunks
NB = 4              # batches per input DMA (4 MiB transfers)

_CACHE = {}
LAST_RESULT = None

_AXON_PJRT_SO = "/opt/axon/libaxon_pjrt.so"


def _ensure_ntff_hook():
    """Provide antenv.axon_hooks if the image lacks it.

    concourse.bass_utils unconditionally imports
    antenv.axon_hooks.get_axon_ntff_profile_hook when trace=True under
    axon; some images ship antenv without that submodule. Register a
    synthetic module wired to libaxon_pjrt.so's NRT-profile C ABI (or a
    None hook, which bass_utils degrades on gracefully).
    """
    import sys
    try:
        from antenv.axon_hooks import get_axon_ntff_profile_hook  # noqa: F401
        return
    except ImportError:
        pass
    import contextlib
    import ctypes
    import types

    hook = None
    try:
        lib = ctypes.CDLL(_AXON_PJRT_SO)
        if hasattr(lib, "axon_start_nrt_profile"):
            lib.axon_start_nrt_profile.argtypes = [
                ctypes.POINTER(ctypes.c_int64), ctypes.c_size_t]
            lib.axon_start_nrt_profile.restype = ctypes.c_int64
            lib.axon_stop_nrt_profile.argtypes = [ctypes.c_char_p]
            lib.axon_stop_nrt_profile.restype = ctypes.c_int64

            @contextlib.contextmanager
            def _hook(output_dir, device_ids):
                import jax
                jax.devices()
                if device_ids:
                    ids = (ctypes.c_int64 * len(device_ids))(*device_ids)
                    rc = lib.axon_start_nrt_profile(ids, len(device_ids))
                else:
                    rc = lib.axon_start_nrt_profile(None, 0)
                if rc != 0:
                    raise RuntimeError(f"axon_start_nrt_profile rc={rc}")
                try:
                    yield
                finally:
                    n = lib.axon_stop_nrt_profile(str(output_dir).encode())
                    if n <= 0:
                        import sys as _s
                        print(f"profile: rc={n} writing {output_dir}",
                              file=_s.stderr)

            hook = _hook
    except OSError:
        pass

    mod = types.ModuleType("antenv.axon_hooks")
    _state = {"hook": hook}
    mod.get_axon_ntff_profile_hook = lambda: _state["hook"]
    mod.set_axon_ntff_profile_hook = lambda h: _state.__setitem__("hook", h)
    sys.modules["antenv.axon_hooks"] = mod
    try:
        import antenv
        antenv.axon_hooks = mod
    except ImportError:
        pass


def _build_nc():
    import concourse.bass as bass  # noqa: F401
    import concourse.tile as tile
    from concourse import bacc, mybir

    f32 = mybir.dt.float32
    nc = bacc.Bacc("TRN2", target_bir_lowering=False, debug=False,
                   num_devices=NCORES)

    x_d = nc.dram_tensor("inputs", [BLOC, M, I], f32, kind="ExternalInput").ap()
    w_d = nc.dram_tensor("w_pci", [128, C, I], f32, kind="ExternalInput").ap()
    b_d = nc.dram_tensor("bias_pc", [128, C], f32, kind="ExternalInput").ap()
    out_d = nc.dram_tensor("out", [BLOC, M], f32, kind="ExternalOutput").ap()
    ident_d = nc.inline_tensor(np.eye(128, dtype=np.float32), name="ident")

    mult = mybir.AluOpType.mult
    add = mybir.AluOpType.add
    ident_fn = mybir.ActivationFunctionType.Identity

    with tile.TileContext(nc) as tc:
        with tc.tile_pool(name="const", bufs=1) as cpool, \
             tc.tile_pool(name="xin", bufs=3) as xpool, \
             tc.tile_pool(name="prodp", bufs=4) as ppool, \
             tc.tile_pool(name="scrp", bufs=4) as spool, \
             tc.tile_pool(name="accp", bufs=1) as apool, \
             tc.tile_pool(name="outp", bufs=1) as opool, \
             tc.tile_pool(name="pprd", bufs=1, space="PSUM") as pppool, \
             tc.tile_pool(name="tpp", bufs=2, space="PSUM") as tppool:

            w_sb = cpool.tile([128, C, I], f32, name="w_sb")
            nc.scalar.dma_start(w_sb[:], w_d[:])
            bias_sb = cpool.tile([128, 1, C], f32, name="bias_sb")
            nc.scalar.dma_start(bias_sb[:, 0], b_d[:])
            ident_sb = cpool.tile([128, 128], f32, name="ident_sb")
            nc.scalar.dma_start(ident_sb[:], ident_d.ap())

            # results land here as [m_p, b, c]
            acc = apool.tile([128, BLOC, C], f32, name="acc")

            # [b, (c p), i] -> [p, b, c, i]
            xv = x_d.rearrange("b (p c) i -> p b c i", p=128)

            blocks = [(0, 1), (1, 3)] + [(k, NB) for k in range(NB, BLOC, NB)]
            for b0, nb in blocks:
                xt = xpool.tile([128, nb, C, I], f32, name="xt", tag="xt")
                nc.sync.dma_start(xt[:], xv[:, b0:b0 + nb])
                for j in range(nb):
                    b = b0 + j
                    if b % 9 not in (0, 2, 4, 6):
                        # DVE-only path: mult + segmented reduce
                        prodd = ppool.tile([128, C, I], f32, name="prodd",
                                           tag="prod")
                        nc.vector.tensor_mul(prodd[:], xt[:, j], w_sb[:])
                        nc.vector.tensor_reduce(
                            out=acc[:, b],
                            in_=prodd[:],
                            axis=mybir.AxisListType.X,
                            op=add,
                        )
                    else:
                        # DVE multiplies into PSUM, ScalarE reduces
                        # (PSUM source shaves ~50 cyc per ACTIVATE)
                        prod = pppool.tile([128, C, I], f32, name="proda",
                                           tag="proda")
                        nc.vector.tensor_mul(prod[:], xt[:, j], w_sb[:])
                        for c in range(C):
                            scr2 = spool.tile([128, I], f32, name="scr2",
                                              tag="scr2")
                            nc.scalar.activation(
                                out=scr2[:],
                                in_=prod[:, c],
                                func=ident_fn,
                                bias=0.0,
                                scale=1.0,
                                accum_out=acc[:, b, c:c + 1],
                            )

            # bias for every batch in one broadcast add
            nc.vector.tensor_add(
                acc[:], acc[:], bias_sb[:].broadcast_to([128, BLOC, C]))

            # acc [m_p, b, c] -> out_sb [b_p, c, m'] via PE transposes
            out_sb = opool.tile([128, 128, C], f32, name="out_sb")
            for c in range(C):
                tp = tppool.tile([128, 128], f32, name="tp", tag="tp")
                nc.tensor.transpose(tp[:], acc[:, :, c], ident_sb[:])
                nc.vector.tensor_copy(out_sb[:, :, c], tp[:])
            nc.sync.dma_start(out_d.rearrange("b (p c) -> b p c", p=128),
                              out_sb[:])

    nc.compile()
    return nc


def _get_nc():
    if "nc" not in _CACHE:
        _CACHE["nc"] = _build_nc()
    return _CACHE["nc"]


def kernel(inputs, Rk_weight, bias):
    global LAST_RESULT
    _ensure_ntff_hook()
    from concourse.bass_utils import run_bass_kernel_spmd

    nc = _get_nc()

    inputs = np.ascontiguousarray(inputs, dtype=np.float32)
    w_pci = np.ascontiguousarray(
        Rk_weight.reshape(128, C, I), dtype=np.float32)
    bias_pc = np.ascontiguousarray(
        bias.reshape(128, C), dtype=np.float32)

    in_maps = []
    for core in range(NCORES):
        in_maps.append({
            "inputs": inputs[core * BLOC:(core + 1) * BLOC],
            "w_pci": w_pci,
            "bias_pc": bias_pc,
        })

    res = run_bass_kernel_spmd(nc, in_maps, list(range(NCORES)))
    LAST_RESULT = res
    out = np.concatenate(
        [np.asarray(res.results[i]["out"]) for i in range(NCORES)], axis=0)
    return out.astype(np.float32, copy=False)


# revision 14
# speedup vs baseline: 1.4838x; 1.4838x over previous
"""Trainium2 Bass kernel for nn_AutoReconstruction.

Computes out[b, m] = dot(inputs[b, m, :], W[m, :]) + bias[m]
  inputs: [1024, 2048, 128] f32, W: [2048, 128] f32, bias: [2048] f32
  out:    [1024, 2048] f32

Sharding: batch dim B=1024 split across 8 NeuronCores (128 each); W/bias
replicated. Per-core traffic ~130 MiB -> memory-bound (~360 GB/s/core).

Per-core algorithm (M chunked p-major: m = p*16 + c, so each partition
reads contiguous 8 KB/batch and DMA descriptors stay large):
  - X loaded 4 batches per DMA as SBUF tiles [p=128, nb, c=16, i=128]
  - per batch, DVE computes prod = X*W in one [128, 2048] tensor_mul;
    the i-reduction is split to keep both engines at ~440 us:
      5/9 of batches: DVE tensor_reduce (axis=X) -> acc[:, b, :]
      4/9 of batches: 16x ScalarE activation(accum_out) chunks
  - one broadcast tensor_add applies bias to acc[m_p, b, c] for all b
  - 16 PE transposes flip acc to [b_p, m]; one contiguous 1 MiB out DMA
Measured: ~483 us/core (DVE ~440, ACT ~440, DMA ~365 active @375 GB/s).
fp32 throughout (rel err ~2e-7). Notes: tensor_tensor_reduce faults on
this runtime; PE fp32 matmul/transpose is ~2.5-5 cyc/col, too slow to
host the reduction.
"""

import numpy as np

B, M, I = 1024, 2048, 128
NCORES = 8
BLOC = B // NCORES  # 128 batches per core
C = M // 128        # 16 m-chunks
NB = 4              # batches per input DMA (4 MiB transfers)

_CACHE = {}
LAST_RESULT = None

_AXON_PJRT_SO = "/opt/axon/libaxon_pjrt.so"


def _ensure_ntff_hook():
    """Provide antenv.axon_hooks if the image lacks it.

    concourse.bass_utils unconditionally imports
    antenv.axon_hooks.get_axon_ntff_profile_hook when trace=True under
    axon; some images ship antenv without that submodule. Register a
    synthetic module wired to libaxon_pjrt.so's NRT-profile C ABI (or a
    None hook, which bass_utils degrades on gracefully).
    """
    import sys
    try:
        from antenv.axon_hooks import get_axon_ntff_profile_hook  # noqa: F401
        return
    except ImportError:
        pass
    import contextlib
    import ctypes
    import types

    hook = None
    try:
        lib = ctypes.CDLL(_AXON_PJRT_SO)
        if hasattr(lib, "axon_start_nrt_profile"):
            lib.axon_start_nrt_profile.argtypes = [
                ctypes.POINTER(ctypes.c_int64), ctypes.c_size_t]
            lib.axon_start_nrt_profile.restype = ctypes.c_int64
            lib.axon_stop_nrt_profile.argtypes = [ctypes.c_char_p]
            lib.axon_stop_nrt_profile.restype = ctypes.c_int64

            @contextlib.contextmanager
            def _hook(output_dir, device_ids):
                import jax
                jax.devices()
                if device_ids:
                    ids = (ctypes.c_int64 * len(device_ids))(*device_ids)
                    rc = lib.axon_start_nrt_profile(ids, len(device_ids))
                else:
                    rc = lib.axon_start_nrt_profile(None, 0)
                if rc != 0:
                    raise RuntimeError(f"axon_start_nrt_profile rc={rc}")
                try:
                    yield
                finally:
                    n = lib.axon_stop_nrt_profile(str(output_dir).encode())
                    if n <= 0:
                        import sys as _s
                        print(f"profile: rc={n} writing {output_dir}",
                              file=_s.stderr)

            hook = _hook
    except OSError:
        pass

    mod = types.ModuleType("antenv.axon_hooks")
    _state = {"hook": hook}
    mod.get_axon_ntff_profile_hook = lambda: _state["hook"]
    mod.set_axon_ntff_profile_hook = lambda h: _state.__setitem__("hook", h)
    sys.modules["antenv.axon_hooks"] = mod
    try:
        import antenv
        antenv.axon_hooks = mod
    except ImportError:
        pass


def _build_nc():
    import concourse.bass as bass  # noqa: F401
    import concourse.tile as tile
    from concourse import bacc, mybir

    f32 = mybir.dt.float32
    bf16 = mybir.dt.bfloat16
    nc = bacc.Bacc("TRN2", target_bir_lowering=False, debug=False,
                   num_devices=NCORES)

    x_d = nc.dram_tensor("inputs", [BLOC, M, I], f32, kind="ExternalInput").ap()
    w_d = nc.dram_tensor("w_pci", [128, C, I], f32, kind="ExternalInput").ap()
    b_d = nc.dram_tensor("bias_pc", [128, C], f32, kind="ExternalInput").ap()
    out_d = nc.dram_tensor("out", [BLOC, M], f32, kind="ExternalOutput").ap()
    ident_d = nc.inline_tensor(np.eye(128, dtype=np.float32), name="ident")

    mult = mybir.AluOpType.mult
    add = mybir.AluOpType.add
    ident_fn = mybir.ActivationFunctionType.Identity

    with tile.TileContext(nc) as tc:
        with tc.tile_pool(name="const", bufs=1) as cpool, \
             tc.tile_pool(name="xin", bufs=3) as xpool, \
             tc.tile_pool(name="prodp", bufs=4) as ppool, \
             tc.tile_pool(name="scrp", bufs=4) as spool, \
             tc.tile_pool(name="accp", bufs=1) as apool, \
             tc.tile_pool(name="outp", bufs=1) as opool, \
             tc.tile_pool(name="tpp", bufs=2, space="PSUM") as tppool:

            w_sb = cpool.tile([128, C, I], f32, name="w_sb")
            nc.scalar.dma_start(w_sb[:], w_d[:])
            bias_sb = cpool.tile([128, 1, C], f32, name="bias_sb")
            nc.scalar.dma_start(bias_sb[:, 0], b_d[:])
            ident_sb = cpool.tile([128, 128], f32, name="ident_sb")
            nc.scalar.dma_start(ident_sb[:], ident_d.ap())

            # results land here as [m_p, b, c]
            acc = apool.tile([128, BLOC, C], f32, name="acc")

            # [b, (c p), i] -> [p, b, c, i]
            xv = x_d.rearrange("b (p c) i -> p b c i", p=128)

            blocks = [(0, 1), (1, 3)] + [(k, NB) for k in range(NB, BLOC, NB)]
            for b0, nb in blocks:
                xt = xpool.tile([128, nb, C, I], f32, name="xt", tag="xt")
                nc.sync.dma_start(xt[:], xv[:, b0:b0 + nb])
                for j in range(nb):
                    b = b0 + j
                    if b % 9 not in (0, 2, 4, 6):
                        # DVE-only path: mult + segmented reduce
                        prodd = ppool.tile([128, C, I], f32, name="prodd",
                                           tag="prod")
                        nc.vector.tensor_mul(prodd[:], xt[:, j], w_sb[:])
                        nc.vector.tensor_reduce(
                            out=acc[:, b],
                            in_=prodd[:],
                            axis=mybir.AxisListType.X,
                            op=add,
                        )
                    else:
                        # DVE multiplies, ScalarE reduces
                        prod = ppool.tile([128, C, I], f32, name="prod",
                                          tag="prod")
                        nc.vector.tensor_mul(prod[:], xt[:, j], w_sb[:])
                        for c in range(C):
                            scr2 = spool.tile([128, I], bf16, name="scr2",
                                              tag="scr2")
                            nc.scalar.activation(
                                out=scr2[:],
                                in_=prod[:, c],
                                func=ident_fn,
                                bias=0.0,
                                scale=1.0,
                                accum_out=acc[:, b, c:c + 1],
                            )

            # bias for every batch in one broadcast add
            nc.vector.tensor_add(
                acc[:], acc[:], bias_sb[:].broadcast_to([128, BLOC, C]))

            # acc [m_p, b, c] -> out_sb [b_p, c, m'] via PE transposes
            out_sb = opool.tile([128, 128, C], f32, name="out_sb")
            for c in range(C):
                tp = tppool.tile([128, 128], f32, name="tp", tag="tp")
                nc.tensor.transpose(tp[:], acc[:, :, c], ident_sb[:])
                nc.vector.tensor_copy(out_sb[:, :, c], tp[:])
            nc.sync.dma_start(out_d.rearrange("b (p c) -> b p c", p=128),
                              out_sb[:])

    nc.compile()
    return nc


def _get_nc():
    if "nc" not in _CACHE:
        _CACHE["nc"] = _build_nc()
    return _CACHE["nc"]


def kernel(inputs, Rk_weight, bias):
    global LAST_RESULT
    _ensure_ntff_hook()
    from concourse.bass_utils import run_bass_kernel_spmd

    nc = _get_nc()

    inputs = np.ascontiguousarray(inputs, dtype=np.float32)
    w_pci = np.ascontiguousarray(
        Rk_weight.reshape(128, C, I), dtype=np.float32)
    bias_pc = np.ascontiguousarray(
        bias.reshape(128, C), dtype=np.float32)

    in_maps = []
    for core in range(NCORES):
        in_maps.append({
            "inputs": inputs[core * BLOC:(core + 1) * BLOC],
            "w_pci": w_pci,
            "bias_pc": bias_pc,
        })

    res = run_bass_kernel_spmd(nc, in_maps, list(range(NCORES)))
    LAST_RESULT = res
    out = np.concatenate(
        [np.asarray(res.results[i]["out"]) for i in range(NCORES)], axis=0)
    return out.astype(np.float32, copy=False)


# revision 16
# speedup vs baseline: 1.4979x; 1.0095x over previous
"""Trainium2 Bass kernel for nn_AutoReconstruction.

Computes out[b, m] = dot(inputs[b, m, :], W[m, :]) + bias[m]
  inputs: [1024, 2048, 128] f32, W: [2048, 128] f32, bias: [2048] f32
  out:    [1024, 2048] f32

Sharding: batch dim B=1024 split across 8 NeuronCores (128 each); W/bias
replicated. Per-core traffic ~130 MiB -> memory-bound (~360 GB/s/core).

Per-core algorithm (M chunked p-major: m = p*16 + c, so each partition
reads contiguous 8 KB/batch and DMA descriptors stay large):
  - X loaded 4 batches per DMA as SBUF tiles [p=128, nb, c=16, i=128]
  - per batch, DVE computes prod = X*W in one [128, 2048] tensor_mul;
    the i-reduction is split to keep both engines at ~440 us:
      5/9 of batches: DVE tensor_reduce (axis=X) -> acc[:, b, :]
      4/9 of batches: 16x ScalarE activation(accum_out) chunks
  - one broadcast tensor_add applies bias to acc[m_p, b, c] for all b
  - 16 PE transposes flip acc to [b_p, m]; one contiguous 1 MiB out DMA
Measured: ~483 us/core (DVE ~440, ACT ~440, DMA ~365 active @375 GB/s).
fp32 throughout (rel err ~2e-7). Notes: tensor_tensor_reduce faults on
this runtime; PE fp32 matmul/transpose is ~2.5-5 cyc/col, too slow to
host the reduction.
"""

import numpy as np

B, M, I = 1024, 2048, 128
NCORES = 8
BLOC = B // NCORES  # 128 batches per core
C = M // 128        # 16 m-chunks
NB = 4              # batches per input DMA (4 MiB transfers)

_CACHE = {}
LAST_RESULT = None

_AXON_PJRT_SO = "/opt/axon/libaxon_pjrt.so"


def _ensure_ntff_hook():
    """Provide antenv.axon_hooks if the image lacks it.

    concourse.bass_utils unconditionally imports
    antenv.axon_hooks.get_axon_ntff_profile_hook when trace=True under
    axon; some images ship antenv without that submodule. Register a
    synthetic module wired to libaxon_pjrt.so's NRT-profile C ABI (or a
    None hook, which bass_utils degrades on gracefully).
    """
    import sys
    try:
        from antenv.axon_hooks import get_axon_ntff_profile_hook  # noqa: F401
        return
    except ImportError:
        pass
    import contextlib
    import ctypes
    import types

    hook = None
    try:
        lib = ctypes.CDLL(_AXON_PJRT_SO)
        if hasattr(lib, "axon_start_nrt_profile"):
            lib.axon_start_nrt_profile.argtypes = [
                ctypes.POINTER(ctypes.c_int64), ctypes.c_size_t]
            lib.axon_start_nrt_profile.restype = ctypes.c_int64
            lib.axon_stop_nrt_profile.argtypes = [ctypes.c_char_p]
            lib.axon_stop_nrt_profile.restype = ctypes.c_int64

            @contextlib.contextmanager
            def _hook(output_dir, device_ids):
                import jax
                jax.devices()
                if device_ids:
                    ids = (ctypes.c_int64 * len(device_ids))(*device_ids)
                    rc = lib.axon_start_nrt_profile(ids, len(device_ids))
                else:
                    rc = lib.axon_start_nrt_profile(None, 0)
                if rc != 0:
                    raise RuntimeError(f"axon_start_nrt_profile rc={rc}")
                try:
                    yield
                finally:
                    n = lib.axon_stop_nrt_profile(str(output_dir).encode())
                    if n <= 0:
                        import sys as _s
                        print(f"profile: rc={n} writing {output_dir}",
                              file=_s.stderr)

            hook = _hook
    except OSError:
        pass

    mod = types.ModuleType("antenv.axon_hooks")
    _state = {"hook": hook}
    mod.get_axon_ntff_profile_hook = lambda: _state["hook"]
    mod.set_axon_ntff_profile_hook = lambda h: _state.__setitem__("hook", h)
    sys.modules["antenv.axon_hooks"] = mod
    try:
        import antenv
        antenv.axon_hooks = mod
    except ImportError:
        pass


def _build_nc():
    import concourse.bass as bass  # noqa: F401
    import concourse.tile as tile
    from concourse import bacc, mybir

    f32 = mybir.dt.float32
    nc = bacc.Bacc("TRN2", target_bir_lowering=False, debug=False,
                   num_devices=NCORES)

    x_d = nc.dram_tensor("inputs", [BLOC, M, I], f32, kind="ExternalInput").ap()
    w_d = nc.dram_tensor("w_pci", [128, C, I], f32, kind="ExternalInput").ap()
    b_d = nc.dram_tensor("bias_pc", [128, C], f32, kind="ExternalInput").ap()
    out_d = nc.dram_tensor("out", [BLOC, M], f32, kind="ExternalOutput").ap()
    ident_d = nc.inline_tensor(np.eye(128, dtype=np.float32), name="ident")

    mult = mybir.AluOpType.mult
    add = mybir.AluOpType.add
    ident_fn = mybir.ActivationFunctionType.Identity

    with tile.TileContext(nc) as tc:
        with tc.tile_pool(name="const", bufs=1) as cpool, \
             tc.tile_pool(name="xin", bufs=4) as xpool, \
             tc.tile_pool(name="prodp", bufs=4) as ppool, \
             tc.tile_pool(name="scrp", bufs=4) as spool, \
             tc.tile_pool(name="accp", bufs=1) as apool, \
             tc.tile_pool(name="outp", bufs=1) as opool, \
             tc.tile_pool(name="tpp", bufs=2, space="PSUM") as tppool:

            w_sb = cpool.tile([128, C, I], f32, name="w_sb")
            nc.scalar.dma_start(w_sb[:], w_d[:])
            bias_sb = cpool.tile([128, 1, C], f32, name="bias_sb")
            nc.scalar.dma_start(bias_sb[:, 0], b_d[:])
            ident_sb = cpool.tile([128, 128], f32, name="ident_sb")
            nc.scalar.dma_start(ident_sb[:], ident_d.ap())

            # results land here as [m_p, b, c]
            acc = apool.tile([128, BLOC, C], f32, name="acc")

            # [b, (c p), i] -> [p, b, c, i]
            xv = x_d.rearrange("b (p c) i -> p b c i", p=128)

            blocks = [(0, 1), (1, 3)] + [(k, NB) for k in range(NB, BLOC, NB)]
            for b0, nb in blocks:
                xt = xpool.tile([128, nb, C, I], f32, name="xt", tag="xt")
                nc.sync.dma_start(xt[:], xv[:, b0:b0 + nb])
                for j in range(nb):
                    b = b0 + j
                    if b % 9 not in (0, 2, 4, 6):
                        # DVE-only path: mult + segmented reduce
                        prodd = ppool.tile([128, C, I], f32, name="prodd",
                                           tag="prod")
                        nc.vector.tensor_mul(prodd[:], xt[:, j], w_sb[:])
                        nc.vector.tensor_reduce(
                            out=acc[:, b],
                            in_=prodd[:],
                            axis=mybir.AxisListType.X,
                            op=add,
                        )
                    else:
                        # DVE multiplies, ScalarE reduces
                        prod = ppool.tile([128, C, I], f32, name="prod",
                                          tag="prod")
                        nc.vector.tensor_mul(prod[:], xt[:, j], w_sb[:])
                        for c in range(C):
                            scr2 = spool.tile([128, I], f32, name="scr2",
                                              tag="scr2")
                            nc.scalar.activation(
                                out=scr2[:],
                                in_=prod[:, c],
                                func=ident_fn,
                                bias=0.0,
                                scale=1.0,
                                accum_out=acc[:, b, c:c + 1],
                            )

            # bias for every batch in one broadcast add
            nc.vector.tensor_add(
                acc[:], acc[:], bias_sb[:].broadcast_to([128, BLOC, C]))

            # acc [m_p, b, c] -> out_sb [b_p, c, m'] via PE transposes
            out_sb = opool.tile([128, 128, C], f32, name="out_sb")
            for c in range(C):
                tp = tppool.tile([128, 128], f32, name="tp", tag="tp")
                nc.tensor.transpose(tp[:], acc[:, :, c], ident_sb[:])
                nc.vector.tensor_copy(out_sb[:, :, c], tp[:])
            odv = out_d.rearrange("b (p c) -> b p c", p=128)
            nc.sync.dma_start(odv[:, 0:64], out_sb[:, 0:64])
            nc.sync.dma_start(odv[:, 64:128], out_sb[:, 64:128])

    nc.compile()
    return nc


def _get_nc():
    if "nc" not in _CACHE:
        _CACHE["nc"] = _build_nc()
    return _CACHE["nc"]


def kernel(inputs, Rk_weight, bias):
    global LAST_RESULT
    _ensure_ntff_hook()
    from concourse.bass_utils import run_bass_kernel_spmd

    nc = _get_nc()

    inputs = np.ascontiguousarray(inputs, dtype=np.float32)
    w_pci = np.ascontiguousarray(
        Rk_weight.reshape(128, C, I), dtype=np.float32)
    bias_pc = np.ascontiguousarray(
        bias.reshape(128, C), dtype=np.float32)

    in_maps = []
    for core in range(NCORES):
        in_maps.append({
            "inputs": inputs[core * BLOC:(core + 1) * BLOC],
            "w_pci": w_pci,
            "bias_pc": bias_pc,
        })

    res = run_bass_kernel_spmd(nc, in_maps, list(range(NCORES)))
    LAST_RESULT = res
    out = np.concatenate(
        [np.asarray(res.results[i]["out"]) for i in range(NCORES)], axis=0)
    return out.astype(np.float32, copy=False)
